# revision 17
# baseline (speedup 1.0000x reference)
"""
MinibatchDiscrimination kernel for 8x TRN2 NeuronCores (Bass/Tile).

Math:  x = inputs @ T  -> [B, K, D] with B=512, K=100, D=5
       out[a,k] = sum_b exp(-sum_d |x[a,k,d]-x[b,k,d]|)

v2 strategy — circulant-symmetric pair coverage (vs v1's full 512-wide rows):

  The distance matrix is symmetric, so each unordered pair {a, b} is computed
  ONCE globally. Core c owns global rows a = 64c+j (j=0..63) and, for each
  row, only the offsets delta = 0..256 (i.e. partners b = a+delta mod 512).
  Every pair {a, a+delta} (delta 1..255) appears exactly once this way;
  delta=256 pairs appear twice (once from each endpoint), handled by
  excluding delta=256 from the row's own accumulation (host subtracts the
  saved dumpc column) while the cross path keeps it.

  Each computed exp(-d(a, a+delta)) contributes to BOTH endpoint rows:
    - row a: ACT accum_out over the row's 257-wide window
    - row a+delta: accumulated into cross[k, j+delta] by the GPSIMD engine
  The host merges: out[64c+j] = raw[:,j] - dumpc[:,j], then scatters
  out[(64c+t) % 512] += cross[:, t] for t = 1..319.

  Per-core inputs are the batch-rolled inputs (roll by -64c), so the program
  is SPMD-identical; only columns 0..319 of the rolled x are needed, so the
  host sends inT = rolled.T[:, 0:320] and the projection matmuls run with
  free size 320 instead of 512.

  Identity used per (k,d):  |x_i - x_j| = 2*relu(x_i - x_j) - x_i + x_j,
  so  sum_d |..| = 2R - S_ki + S_kj  with S = sum_d x.  The -S_ki term is a
  single negI matmul into the dist psum (start of the accumulation group);
  the +S_kj term is constant per row and folded into the exp bias AP
  (bias = -S_kj, scale = -1), eliminating v1's final multiply.

  Per row j (engine assignment tuned against the CoreSim cost model, which
  charges matmuls as out_free_size x 0.42ns serialized on PE):
    PE : negI matmul (dist = -S window, FD=257) + 4 d-sum ones-matmuls
    DVE: 4x tensor_scalar relu chunks [125, 257] (fp16, 4x mode)
         + dumpc column save [128,1]
    ACT: exp(-dist - S_kj) -> dump (SBUF fp16) + accum_out -> raw psum
    POOL: cross[:, j+1..j+257] += dump[:, 1..257]   (the idle gpsimd engine
          takes the symmetric-partner accumulation)

  dist psum layout: partition 32c+m holds k=25c+m (m<25); host reassembles.
"""

import sys
import numpy as np

for _p in ("/opt/trn_rl_repo",):
    if _p not in sys.path:
        sys.path.insert(0, _p)

B = 512
F = 1024
K = 100
D = 5
KD = K * D  # 500
NCORES = 8
JPC = B // NCORES  # 64 output rows per core
NCHUNK = 4  # kd chunks of 125
CHUNK = KD // NCHUNK  # 125
KPC = K // NCHUNK  # 25 k's per chunk
FD = 257  # per-row window: delta = 0..256
W = JPC + FD - 1  # 320 columns of x needed per core

_NC_CACHE = {}


def build_nc():
    import contextlib

    import concourse.bass as bass
    import concourse.bacc as bacc
    import concourse.mybir as mybir
    from concourse.tile import TileContext

    nc = bacc.Bacc(None, target_bir_lowering=False, debug=True)

    inT = nc.declare_dram_parameter("inT", [F, W], mybir.dt.float16, isOutput=False)
    Tm = nc.declare_dram_parameter("Tm", [F, KD], mybir.dt.float16, isOutput=False)
    # [:, 0:32] 2.0-valued d-sum block, [:, 32:64] 1.0-valued d-sum block
    onesd = nc.declare_dram_parameter(
        "onesd", [CHUNK, 64], mybir.dt.float16, isOutput=False
    )
    negI = nc.declare_dram_parameter("negI", [128, 128], mybir.dt.float16, isOutput=False)
    raw_out = nc.declare_dram_parameter("raw", [128, JPC], mybir.dt.float32, isOutput=True)
    cross_out = nc.declare_dram_parameter(
        "cross", [128, W], mybir.dt.float32, isOutput=True
    )

    with TileContext(nc) as tc:
        with tc.tile_pool(name="persist", bufs=1) as pp:
            T_sb = pp.tile([128, 8 * KD], mybir.dt.float16, name="T_sb")
            inT_sb = pp.tile([128, 8 * W], mybir.dt.float16, name="inT_sb")
            ones_sb = pp.tile([CHUNK, 64], mybir.dt.float16, name="ones_sb")
            negI_sb = pp.tile([128, 128], mybir.dt.float16, name="negI_sb")
            xT_sb = pp.tile([128, NCHUNK * W], mybir.dt.float16, name="xT_sb")
            # f32 upcasts of the fp16 xT columns 0..JPC (tensor_scalar
            # per-partition scalars must be f32; upcasting from the fp16 xT
            # keeps the diagonal subtract exactly zero).
            xTj_sb = pp.tile([128, NCHUNK * JPC], mybir.dt.float32, name="xTj_sb")
            S16_sb = pp.tile([128, W], mybir.dt.float16, name="S16_sb")
            negS32_sb = pp.tile([128, JPC], mybir.dt.float32, name="negS32_sb")
            cross_sb = pp.tile([128, W], mybir.dt.float32, name="cross_sb")
            raw_sb = pp.tile([128, JPC], mybir.dt.float32, name="raw_sb")
            dumpc_sb = pp.tile([128, JPC], mybir.dt.float32, name="dumpc_sb")
            dump_bufs = [
                pp.tile([128, FD], mybir.dt.float16, name=f"dump{i}") for i in range(4)
            ]
            NAB = 8
            ab_bufs = [
                pp.tile([CHUNK, FD], mybir.dt.float16, name=f"ab{i}") for i in range(NAB)
            ]

            # warm the ACT exp table while DMAs run (table load ~1.3us)
            warm_sb = pp.tile([1, 1], mybir.dt.float32, name="warm_sb")
            nc.vector.memset(warm_sb[:, :], 0.0)
            nc.scalar.activation(
                warm_sb[:, :], warm_sb[:, :], mybir.ActivationFunctionType.Exp
            )
            nc.vector.memset(cross_sb[:, :], 0.0)

            # --- load inputs (one DMA per tensor; fewer DMAs = less SP/DGE
            # serialization in the cost model) ---
            for h in range(4):
                nc.sync.dma_start(
                    out=T_sb[:, h * 2 * KD : (h + 1) * 2 * KD],
                    in_=Tm[h * 256 : (h + 1) * 256, :].rearrange(
                        "(t p) c -> p t c", t=2
                    ),
                )
                nc.sync.dma_start(
                    out=inT_sb[:, h * 2 * W : (h + 1) * 2 * W],
                    in_=inT[h * 256 : (h + 1) * 256, :].rearrange(
                        "(t p) c -> p t c", t=2
                    ),
                )
                if h == 0:
                    nc.sync.dma_start(out=ones_sb[:, :], in_=onesd[:, :])
                    nc.sync.dma_start(out=negI_sb[:, :], in_=negI[:, :])

            with tc.tile_pool(name="xtps", bufs=1, space="PSUM") as xtps:
                # --- xT chunks: xT[kd, i] via PE over f tiles, window W.
                # t-outer order so the first f-tiles' matmuls overlap the
                # second half of the input DMAs. ---
                xt_ps = [
                    xtps.tile([CHUNK, W], mybir.dt.float32, name=f"xt_ps{c}")
                    for c in range(NCHUNK)
                ]
                S_ps = xtps.tile([128, W], mybir.dt.float32, name="S_ps", bufs=1)
                # t-outer for tiles 0..5 (runnable as DMA quarters land),
                # then per-chunk tails (t=6,7) so each chunk's psum->sbuf
                # copy starts while the next chunk's tail matmuls run
                for t in range(6):
                    for c in range(NCHUNK):
                        nc.tensor.matmul(
                            xt_ps[c][:, :],
                            T_sb[:, t * KD + c * CHUNK : t * KD + (c + 1) * CHUNK],
                            inT_sb[:, t * W : (t + 1) * W],
                            start=(t == 0),
                            stop=False,
                            skip_group_check=True,
                        )
                for c in range(NCHUNK):
                    for t in (6, 7):
                        nc.tensor.matmul(
                            xt_ps[c][:, :],
                            T_sb[:, t * KD + c * CHUNK : t * KD + (c + 1) * CHUNK],
                            inT_sb[:, t * W : (t + 1) * W],
                            start=False,
                            stop=(t == 7),
                            skip_group_check=True,
                        )
                    nc.vector.tensor_copy(
                        xT_sb[0:CHUNK, c * W : (c + 1) * W], xt_ps[c][:, :]
                    )
                    # xTj upcast on the (idle) ACT engine to keep DVE short
                    nc.scalar.copy(
                        xTj_sb[0:CHUNK, c * JPC : (c + 1) * JPC],
                        xT_sb[0:CHUNK, c * W : c * W + JPC],
                    )
                # --- S[k, i] = sum_d x[i,k,d] from the fp16 xT (so the
                # diagonal cancels exactly), arranged at partitions 32c+m ---
                for c in range(NCHUNK):
                    nc.tensor.matmul(
                        S_ps[32 * c : 32 * c + 32, :],
                        ones_sb[:, 32:64],
                        xT_sb[0:CHUNK, c * W : (c + 1) * W],
                        start=True,
                        stop=True,
                        tile_position=(0, 32 * c),
                    )
                nc.vector.tensor_copy(S16_sb[:, :], S_ps[:, :])
                nc.vector.tensor_scalar(
                    negS32_sb[:, :],
                    S_ps[:, 0:JPC],
                    -1.0,
                    None,
                    mybir.AluOpType.mult,
                )

            mainps_es = contextlib.ExitStack()
            mainps = mainps_es.enter_context(
                tc.tile_pool(name="mainps", bufs=1, space="PSUM")
            )
            # full-bank tiles (512 f32) so no two dist tiles share a psum
            # bank; only [:, 0:FD] is used
            dist_bufs = [
                mainps.tile([128, 512], mybir.dt.float32, name=f"dist{i}")
                for i in range(6)
            ]
            raw_ps_h = [
                mainps.tile([128, JPC // 2], mybir.dt.float32, name=f"raw_ps{h}")
                for h in range(2)
            ]

            # --- main loop over output rows ---
            for j in range(JPC):
                dist = dist_bufs[j % 6]
                for c in range(NCHUNK):
                    ab = ab_bufs[(j * NCHUNK + c) % NAB]
                    # relu(x_i - x_j) = (x_i - x_j) max 0
                    nc.vector.tensor_scalar(
                        ab[:, :],
                        xT_sb[0:CHUNK, c * W + j : c * W + j + FD],
                        xTj_sb[0:CHUNK, c * JPC + j : c * JPC + j + 1],
                        0.0,
                        mybir.AluOpType.subtract,
                        mybir.AluOpType.max,
                    )
                    # dist[32c+m, :] = 2 * sum_d ab[5m+d, :]  (start=True:
                    # each chunk initializes its own 32-partition group; the
                    # -S term lands last so row 0's d-sums don't wait on S16)
                    nc.tensor.matmul(
                        dist[32 * c : 32 * c + 32, 0:FD],
                        ones_sb[:, 0:32],
                        ab[:, :],
                        start=True,
                        stop=False,
                        tile_position=(0, 32 * c),
                        skip_group_check=True,
                    )
                # dist += -S[k, j..j+FD] (closes the group)
                nc.tensor.matmul(
                    dist[:, 0:FD],
                    negI_sb[:, :],
                    S16_sb[:, j : j + FD],
                    start=False,
                    stop=True,
                    skip_group_check=True,
                )
                dump = dump_bufs[j % 4]
                # dump = exp(-dist - S_kj); raw[:, j] = sum_i dump
                nc.scalar.activation(
                    dump[:, :],
                    dist[:, 0:FD],
                    mybir.ActivationFunctionType.Exp,
                    bias=negS32_sb[:, j : j + 1],
                    scale=-1.0,
                    accum_out=raw_ps_h[j // (JPC // 2)][:, j % (JPC // 2) : j % (JPC // 2) + 1],
                )
                # dumpc = computed diag + delta=256 column: both subtracted
                # from raw at the end (host re-adds the exact self term 1.0;
                # delta=256 is owned by the partner row's cross path)
                nc.vector.tensor_tensor(
                    dumpc_sb[:, j : j + 1],
                    dump[:, 0:1],
                    dump[:, FD - 1 : FD],
                    mybir.AluOpType.add,
                )
                # symmetric partners: cross[k, j+delta] += dump[k, delta]
                nc.gpsimd.tensor_tensor(
                    cross_sb[:, j + 1 : j + FD],
                    cross_sb[:, j + 1 : j + FD],
                    dump[:, 1:FD],
                    mybir.AluOpType.add,
                )
                if j == JPC // 2 - 1 or j == JPC - 1:
                    # finalize and ship this half of the own-row sums while
                    # the loop (or the cross DMA) continues
                    h0 = 0 if j < JPC // 2 else JPC // 2
                    nc.vector.tensor_tensor(
                        raw_sb[:, h0 : j + 1],
                        raw_ps_h[j // (JPC // 2)][:, :],
                        dumpc_sb[:, h0 : j + 1],
                        mybir.AluOpType.subtract,
                    )
                    nc.sync.dma_start(
                        out=raw_out[:, h0 : j + 1], in_=raw_sb[:, h0 : j + 1]
                    )

            mainps_es.close()
            nc.sync.dma_start(out=raw_out[:, :], in_=raw_sb[:, :])
            nc.sync.dma_start(out=cross_out[:, :], in_=cross_sb[:, :])

    nc.finalize()
    return nc


def _aux_consts():
    ob = np.zeros([CHUNK, 64], dtype=np.float16)
    for m in range(KPC):
        ob[5 * m : 5 * m + 5, m] = 2.0
        ob[5 * m : 5 * m + 5, 32 + m] = 1.0
    negI = (-np.eye(128)).astype(np.float16)
    return ob, negI


def make_in_maps(inputs, T):
    f16 = np.float16
    Tm = np.asarray(T, dtype=np.float32).astype(f16)
    ob, negI = _aux_consts()
    in_maps = []
    for c in range(NCORES):
        rolled = np.roll(np.asarray(inputs, dtype=np.float32), -JPC * c, axis=0)
        inTc = np.ascontiguousarray(rolled[0:W].T).astype(f16)
        in_maps.append(
            {
                "inT": inTc,
                "Tm": Tm,
                "onesd": ob,
                "negI": negI,
            }
        )
    return in_maps


def assemble_output(results):
    out = np.zeros([B, K], dtype=np.float32)
    for c in range(NCORES):
        own = np.asarray(results[c]["raw"], dtype=np.float32) + 1.0  # [128, JPC]
        cross = np.asarray(results[c]["cross"], dtype=np.float32)  # [128, W]
        for cc in range(NCHUNK):
            ksl = slice(32 * cc, 32 * cc + KPC)
            kg = slice(KPC * cc, KPC * (cc + 1))
            # own rows: global rows 64c..64c+63
            out[JPC * c : JPC * (c + 1), kg] += own[ksl, :].T
            # cross rows: global rows (64c + t) % 512 for t = 1..W-1
            rows = (JPC * c + np.arange(1, W)) % B
            np.add.at(out, (rows[:, None], np.arange(KPC * cc, KPC * (cc + 1))[None, :]),
                      cross[ksl, 1:W].T)
    return out


def kernel(inputs, T):
    from concourse.bass_utils import run_bass_kernel_spmd

    if "nc" not in _NC_CACHE:
        _NC_CACHE["nc"] = build_nc()
    nc = _NC_CACHE["nc"]
    in_maps = make_in_maps(inputs, T)
    res = run_bass_kernel_spmd(nc, in_maps, list(range(NCORES)))
    return assemble_output(res.results)


if __name__ == "__main__":
    sys.path.insert(0, "/root/problem")
    from reference import setup_inputs, reference

    inputs = setup_inputs()
    expected = np.asarray(reference(**inputs))
    actual = kernel(**{k: np.asarray(v) for k, v in inputs.items()})
    err = np.abs(actual - expected)
    rel = np.linalg.norm(actual - expected) / np.linalg.norm(expected)
    print(f"max abs err: {err.max():.3e}")
    print(f"Relative error: {rel:.3e}")
